# revision 13
# baseline (speedup 1.0000x reference)
"""Trainium2 Bass kernel: transformer decoder block (causal self-attention +
cross-attention + 4x FFN, post-residual layernorms).

Sharding: pure data-parallel over batch. B=64 -> 8 batch elements per core on
8 NeuronCores, no collectives. Each core runs the same Bass program on its
batch shard; weights are replicated.

Layout strategy per core:
  - the residual stream stays fp32 token-major [T_part, C_free] (layernorm /
    residual adds are free-dim reductions there), and is transposed on the PE
    (identity matmul) into fp16 feature-major [C_part, T_free] tiles that feed
    the matmuls. All matmul operands are fp16 (PE runs fp16 at full rate and
    upconverts to fp22 internally; accumulation is fp32 in PSUM), produced by
    converting writes of the PSUM->SBUF eviction ops, so the conversions cost
    nothing extra.
  - attention is computed in S^T layout: S^T[k, q] = K^T.T @ Q^T per head; exp
    runs on the scalar engine straight out of PSUM (max-subtraction is not
    needed: |logits| <= ~6 for layernormed inputs); causal masking is a gpsimd
    affine_select (fill=0 after exp); softmax denominators come from a
    ones-lhsT matmul on the PE that produces 64-row replicated column sums;
    the 1/sum normalization is fused into the O^T PSUM->SBUF eviction.
  - weights are converted to fp16 once at startup and stay SBUF-resident.
"""

import os
import sys
from contextlib import ExitStack

import numpy as np

for _p in ("/opt/trn_rl_repo",):
    if os.path.isdir(_p) and _p not in sys.path:
        sys.path.insert(0, _p)

import concourse.bass as bass
import concourse.tile as tile
from concourse import mybir
from concourse import bass_utils
from concourse.masks import make_identity

B, T, C = 64, 312, 512
NH, HD, FF = 8, 64, 2048
N_CORES = 8
BPC = B // N_CORES
NKC = C // 128          # channel chunks
NFC = FF // 128         # ffn-dim chunks
TT = [(0, 128), (128, 128), (256, T - 256)]   # token tiles (t0, sz)
F32 = mybir.dt.float32
F16 = mybir.dt.float16
AL = mybir.AluOpType
AF = mybir.ActivationFunctionType

_WAIT_CAP = 1


def _split_sync_waits(nc):
    """This walrus build supports only one sync-wait command per instruction.
    Redistribute excess waits onto same-engine nops inserted directly before
    the instruction (waits are pure pre-conditions on monotonic semaphores,
    so hoisting them earlier on the same engine preserves ordering; DMA waits
    execute on the triggering sequencer, so the same argument applies)."""
    cap = _WAIT_CAP
    for bb in nc.main_func.blocks:
        il = bb.instructions
        i = 0
        while i < len(il):
            inst = il[i]
            si = inst.sync_info
            if si is None or not si.on_wait or len(si.on_wait) <= cap:
                i += 1
                continue
            waits = list(si.on_wait)
            extra, keep = waits[:-cap], waits[-cap:]
            inst.sync_info = mybir.SyncInfo(on_wait=keep,
                                            on_update=list(si.on_update or []))
            for j in range(0, len(extra), cap):
                nop = mybir.InstNoOp(name=f"I-waitsplit-{nc.next_id()}",
                                     ins=[], outs=[])
                nop.engine = inst.engine
                nop.sync_info = mybir.SyncInfo(on_wait=extra[j:j + cap],
                                               on_update=[])
                il.insert(i, nop)
                i += 1
            i += 1


def _mm(nc, out, lhsT, rhs, start, stop):
    # skip_group_check: CoreSim's psum group checker mis-tracks partition-
    # sliced accumulation groups (base_partition=64); execution semantics
    # (per-element has_written) are unaffected. Tile still provides ordering.
    nc.tensor.matmul(out, lhsT, rhs, start=start, stop=stop,
                     skip_group_check=True)


def _build_program(bpc):
    """Build the per-core Bass program for `bpc` batch elements."""
    nc = bass.Bass("TRN2", target_bir_lowering=False, debug=False,
                   enable_asserts=False, num_devices=N_CORES)

    xd = nc.dram_tensor("x", [bpc, T, C], F32, kind="ExternalInput").ap()
    ed = nc.dram_tensor("enc", [bpc, T, C], F32, kind="ExternalInput").ap()
    wnames = ["wq_sa", "wk_sa", "wv_sa", "wo_sa",
              "wq_ca", "wk_ca", "wv_ca", "wo_ca"]
    wd = {nm: nc.dram_tensor(nm, [C, C], F32, kind="ExternalInput").ap()
          for nm in wnames}
    w1d = nc.dram_tensor("w1", [C, FF], F32, kind="ExternalInput").ap()
    w2d = nc.dram_tensor("w2", [FF, C], F32, kind="ExternalInput").ap()
    outd = nc.dram_tensor("out", [bpc, T, C], F32, kind="ExternalOutput").ap()

    with tile.TileContext(nc) as tc, ExitStack() as ctx:
        con = ctx.enter_context(tc.tile_pool(name="con", bufs=1))
        stg = ctx.enter_context(tc.tile_pool(name="stg", bufs=2))
        act = ctx.enter_context(tc.tile_pool(name="act", bufs=4))
        actT = ctx.enter_context(tc.tile_pool(name="actT", bufs=2))
        qkp = ctx.enter_context(tc.tile_pool(name="qkp", bufs=2))
        vpp = ctx.enter_context(tc.tile_pool(name="vpp", bufs=2))
        esp = ctx.enter_context(tc.tile_pool(name="esp", bufs=6))
        rbp = ctx.enter_context(tc.tile_pool(name="rbp", bufs=2))
        oTp = ctx.enter_context(tc.tile_pool(name="oTp", bufs=2))
        hTp = ctx.enter_context(tc.tile_pool(name="hTp", bufs=NFC))
        sml = ctx.enter_context(tc.tile_pool(name="sml", bufs=3))
        ps_s = ctx.enter_context(tc.tile_pool(name="ps_s", bufs=2, space="PSUM"))
        ps_o = ctx.enter_context(tc.tile_pool(name="ps_o", bufs=2, space="PSUM"))
        ps_cs = ctx.enter_context(tc.tile_pool(name="ps_cs", bufs=2, space="PSUM"))
        ps_mm = ctx.enter_context(tc.tile_pool(name="ps_mm", bufs=2, space="PSUM"))

        # ---- constants & resident fp16 weights (converted via staging) ----
        ws = {}
        for nm in wnames:
            st = stg.tile([128, NKC, C], F32, name=f"{nm}_st", tag="stg")
            nc.sync.dma_start(out=st, in_=wd[nm].rearrange("(kc p) n -> p kc n", p=128))
            wt = con.tile([128, NKC, C], F16, name=f"{nm}_sb", tag=f"{nm}_sb")
            nc.vector.tensor_copy(wt, st)
            ws[nm] = wt
        w2s = con.tile([128, NFC, C], F16)
        w2r = w2d.rearrange("(fc p) n -> p fc n", p=128)
        for g in range(NFC // NKC):
            st = stg.tile([128, NKC, C], F32, name=f"w2_st{g}", tag="stg")
            nc.sync.dma_start(out=st, in_=w2r[:, g * NKC:(g + 1) * NKC, :])
            nc.vector.tensor_copy(w2s[:, g * NKC:(g + 1) * NKC, :], st)
        w1s = con.tile([128, NKC, FF], F16)
        w1r = w1d.rearrange("(kc p) f -> p kc f", p=128)
        for g in range(NFC // NKC):
            st = stg.tile([128, NKC, C], F32, name=f"w1_st{g}", tag="stg")
            nc.sync.dma_start(out=st, in_=w1r[:, :, g * C:(g + 1) * C])
            nc.vector.tensor_copy(w1s[:, :, g * C:(g + 1) * C], st)
        ident = con.tile([128, 128], F32)
        make_identity(nc, ident)
        ones = con.tile([128, 64], F16)
        nc.vector.memset(ones, 1.0)
        epsT = con.tile([128, 1], F32)
        nc.vector.memset(epsT, 1e-5)

        def transpose_tf(src, nm):
            """fp32 token-major [128, 3, C] -> fp16 feature-major [128, NKC, T]."""
            dst = actT.tile([128, NKC, T], F16, name=nm, tag="actT")
            for cc in range(NKC):
                for it, (t0, sz) in enumerate(TT):
                    tp = ps_s.tile([128, T], F32, name=f"{nm}_tp", tag="s")
                    nc.tensor.transpose(
                        tp[:, :sz], src[:sz, it, cc * 128:(cc + 1) * 128],
                        ident[:sz, :sz])
                    nc.vector.tensor_copy(dst[:, cc, t0:t0 + sz], tp[:, :sz])
            return dst

        def proj_feat(srcT, w, nm):
            """Q^T/K^T-style projection: fp16 [128, NKC, T] = w.T @ srcT."""
            dst = qkp.tile([128, NKC, T], F16, name=nm, tag="qk")
            for mc in range(NKC):
                pp = ps_s.tile([128, T], F32, name=f"{nm}_pp", tag="s")
                for kc in range(NKC):
                    _mm(nc, pp, w[:, kc, mc * 128:(mc + 1) * 128],
                        srcT[:, kc, :], (kc == 0), (kc == NKC - 1))
                nc.vector.tensor_copy(dst[:, mc, :], pp)
            return dst

        def proj_tok(srcT, w, nm):
            """V-style projection, fp16 token-major out [128, 3, C]."""
            dst = vpp.tile([128, 3, C], F16, name=nm, tag="v")
            for it, (t0, sz) in enumerate(TT):
                pp = ps_mm.tile([128, C], F32, name=f"{nm}_pp", tag="mm512")
                for kc in range(NKC):
                    _mm(nc, pp[:sz, :], srcT[:, kc, t0:t0 + sz], w[:, kc, :],
                        (kc == 0), (kc == NKC - 1))
                nc.vector.tensor_copy(dst[:sz, it, :], pp[:sz, :])
            return dst

        def attention(qT, kT, v, causal, nm):
            """-> O^T fp16 feature-major [128, NKC, T]: softmax(QK^T/8)V."""
            oT = oTp.tile([128, NKC, T], F16, name=nm, tag="oT")
            css = rbp.tile([128, NKC, T], F32, name=f"{nm}_css", tag="rb")
            steps = [(h, kt) for h in range(NH) for kt in range(3)]

            def emit_s(h, kt):
                k0, ksz = TT[kt]
                q0 = k0 if causal else 0
                qh = qT[(h % 2) * 64:(h % 2) * 64 + 64, h // 2, :]
                kh = kT[(h % 2) * 64:(h % 2) * 64 + 64, h // 2, :]
                s_ps = ps_s.tile([128, T], F32, name=f"{nm}_s{h}_{kt}", tag="s")
                _mm(nc, s_ps[:ksz, q0:T], kh[:, k0:k0 + ksz], qh[:, q0:T],
                    True, True)
                return s_ps

            s_next = emit_s(*steps[0])
            o_ps = cs_ps = None
            for i, (h, kt) in enumerate(steps):
                pr, half = h // 2, (h % 2) * 64
                osl = slice(half, half + 64)
                k0, ksz = TT[kt]
                q0 = k0 if causal else 0
                s_ps = s_next
                if i + 1 < len(steps):
                    s_next = emit_s(*steps[i + 1])
                es = esp.tile([128, T], F16, name=f"{nm}_es{h}_{kt}", tag="es")
                nc.scalar.activation(es[:ksz, q0:T], s_ps[:ksz, q0:T],
                                     AF.Exp, scale=HD ** -0.5)
                if causal:
                    if k0 > 0:
                        nc.vector.memset(es[:ksz, 0:k0], 0.0)
                    nc.gpsimd.affine_select(
                        out=es[:ksz, k0:T], in_=es[:ksz, k0:T],
                        pattern=[[1, T - k0]], channel_multiplier=-1,
                        base=0, compare_op=AL.is_ge, fill=0.0)
                if kt == 0:
                    o_ps = ps_o.tile([128, T], F32, name=f"{nm}_o{h}", tag="o")
                    cs_ps = ps_cs.tile([128, T], F32, name=f"{nm}_cs{h}", tag="cs")
                _mm(nc, o_ps[osl, :], v[:ksz, kt, h * HD:(h + 1) * HD],
                    es[:ksz, :], (kt == 0), (kt == 2))
                _mm(nc, cs_ps[osl, :], ones[:ksz, :], es[:ksz, :],
                    (kt == 0), (kt == 2))
                if kt == 2:
                    nc.vector.tensor_copy(css[osl, pr, :], cs_ps[osl, :])
                    nc.vector.tensor_copy(oT[osl, pr, :], o_ps[osl, :])
            # one Ln + one Exp(-x) on the scalar engine turn all 8 heads'
            # column sums into reciprocals (2 table loads instead of 8 slow
            # DVE RECIPROCALs), then a single DVE multiply normalizes O^T.
            nc.scalar.activation(css, css, AF.Ln)
            nc.scalar.activation(css, css, AF.Exp, scale=-1.0)
            nc.vector.tensor_tensor(out=oT, in0=oT, in1=css, op=AL.mult)
            return oT

        def out_proj_residual(oT, w, res, nm):
            """fp32 token-major [128, 3, C] = oT.T @ w + res."""
            dst = act.tile([128, 3, C], F32, name=nm, tag="act")
            for it, (t0, sz) in enumerate(TT):
                pp = ps_mm.tile([128, C], F32, name=f"{nm}_pp", tag="mm512")
                for pr in range(NKC):
                    _mm(nc, pp[:sz, :], oT[:, pr, t0:t0 + sz], w[:, pr, :],
                        (pr == 0), (pr == NKC - 1))
                nc.vector.tensor_tensor(out=dst[:sz, it, :], in0=pp[:sz, :],
                                        in1=res[:sz, it, :], op=AL.add)
            return dst

        def layernorm_(r, nm):
            """in-place LN over C on token-major [128, 3, C] (g=1, b=0)."""
            for it, (t0, sz) in enumerate(TT):
                stats = sml.tile([128, 6], F32, name=f"{nm}_st", tag="st")
                nc.vector.bn_stats(out=stats[:sz, :], in_=r[:sz, it, :])
                mv = sml.tile([128, 2], F32, name=f"{nm}_mv", tag="mv")
                nc.vector.bn_aggr(out=mv[:sz, :], in_=stats[:sz, :])
                std = sml.tile([128, 1], F32, name=f"{nm}_sd", tag="sd")
                nc.scalar.activation(std[:sz, :], mv[:sz, 1:2], AF.Sqrt,
                                     bias=epsT[:sz, :])
                rstd = sml.tile([128, 1], F32, name=f"{nm}_rs", tag="rs")
                nc.vector.reciprocal(rstd[:sz, :], std[:sz, :])
                nc.gpsimd.tensor_scalar(
                    out=r[:sz, it, :], in0=r[:sz, it, :],
                    scalar1=mv[:sz, 0:1], scalar2=rstd[:sz, :],
                    op0=AL.subtract, op1=AL.mult)
            return r

        for b in range(bpc):
            x_sb = act.tile([128, 3, C], F32, name=f"x{b}", tag="act")
            for it, (t0, sz) in enumerate(TT):
                nc.sync.dma_start(out=x_sb[:sz, it, :], in_=xd[b, t0:t0 + sz, :])
            enc_sb = act.tile([128, 3, C], F32, name=f"e{b}", tag="act")
            for it, (t0, sz) in enumerate(TT):
                nc.sync.dma_start(out=enc_sb[:sz, it, :], in_=ed[b, t0:t0 + sz, :])
            xT = transpose_tf(x_sb, f"xT{b}")
            # ---- self attention ----
            qT = proj_feat(xT, ws["wq_sa"], f"qT{b}")
            kT = proj_feat(xT, ws["wk_sa"], f"kT{b}")
            v = proj_tok(xT, ws["wv_sa"], f"v{b}")
            oT = attention(qT, kT, v, True, f"sa{b}")
            x1 = out_proj_residual(oT, ws["wo_sa"], x_sb, f"r1_{b}")
            # encT is LN1-independent: emitting it here gives the in-order PE
            # stream ready work while the DVE runs LN1.
            encT = transpose_tf(enc_sb, f"eT{b}")
            layernorm_(x1, f"ln1_{b}")
            # ---- cross attention ----
            x1T = transpose_tf(x1, f"x1T{b}")
            qcT = proj_feat(x1T, ws["wq_ca"], f"qcT{b}")
            kcT = proj_feat(encT, ws["wk_ca"], f"kcT{b}")
            vc = proj_tok(encT, ws["wv_ca"], f"vc{b}")
            oTc = attention(qcT, kcT, vc, False, f"ca{b}")
            x2 = out_proj_residual(oTc, ws["wo_ca"], x1, f"r2_{b}")
            layernorm_(x2, f"ln2_{b}")
            # ---- FFN ----
            x2T = transpose_tf(x2, f"x2T{b}")
            hTs = []
            for fc in range(NFC):
                hp = ps_s.tile([128, T], F32, name=f"h{b}_{fc}", tag="s")
                for kc in range(NKC):
                    _mm(nc, hp, w1s[:, kc, fc * 128:(fc + 1) * 128],
                        x2T[:, kc, :], (kc == 0), (kc == NKC - 1))
                hT = hTp.tile([128, T], F16, name=f"hT{b}_{fc}", tag="hT")
                nc.scalar.activation(hT, hp, AF.Relu)
                hTs.append(hT)
            x3 = act.tile([128, 3, C], F32, name=f"r3_{b}", tag="act")
            for it, (t0, sz) in enumerate(TT):
                yp = ps_mm.tile([128, C], F32, name=f"y{b}_{it}", tag="mm512")
                for fc in range(NFC):
                    _mm(nc, yp[:sz, :], hTs[fc][:, t0:t0 + sz], w2s[:, fc, :],
                        (fc == 0), (fc == NFC - 1))
                nc.vector.tensor_tensor(out=x3[:sz, it, :], in0=yp[:sz, :],
                                        in1=x2[:sz, it, :], op=AL.add)
            layernorm_(x3, f"ln3_{b}")
            for it, (t0, sz) in enumerate(TT):
                nc.sync.dma_start(out=outd[b, t0:t0 + sz, :], in_=x3[:sz, it, :])

    return nc


def _np_reference(x, enc_out, min_mask, mout,
                  Wq_sa, Wk_sa, Wv_sa, Wo_sa, bo_sa,
                  Wq_ca, Wk_ca, Wv_ca, Wo_ca, bo_ca,
                  W1, b1, W2, b2, g1, be1, gc, bec, g2, be2):
    """Pure-numpy fallback (exact reference semantics, any inputs)."""
    def ln(x, g, b, eps=1e-5):
        m = x.mean(-1, keepdims=True)
        v = ((x - m) ** 2).mean(-1, keepdims=True)
        return (x - m) / np.sqrt(v + eps) * g + b

    def mha(xq, xkv, Wq, Wk, Wv, Wo, bo, key_mask, causal):
        Bq, Tq, Cc = xq.shape
        Tk = xkv.shape[1]
        q = (xq @ Wq).reshape(Bq, Tq, NH, HD)
        k = (xkv @ Wk).reshape(Bq, Tk, NH, HD)
        vv = (xkv @ Wv).reshape(Bq, Tk, NH, HD)
        wei = np.einsum("bqhd,bkhd->bhqk", q, k) * (HD ** -0.5)
        mask = (key_mask[:, None, None, :] != 0)
        if causal:
            tril = np.tril(np.ones((Tq, Tk), bool))
            mask = mask & tril[None, None]
        wei = np.where(mask, wei, -1e30)
        wei = wei - wei.max(-1, keepdims=True)
        wei = np.exp(wei)
        wei = wei / wei.sum(-1, keepdims=True)
        out = np.einsum("bhqk,bkhd->bqhd", wei, vv).reshape(Bq, Tq, Cc)
        return out @ Wo + bo

    x = x.astype(np.float64)
    att = mha(x, x, Wq_sa, Wk_sa, Wv_sa, Wo_sa, bo_sa, mout, True)
    x = ln(att + x, g1, be1)
    catt = mha(x, enc_out.astype(np.float64), Wq_ca, Wk_ca, Wv_ca, Wo_ca,
               bo_ca, min_mask, False)
    x = ln(catt + x, gc, bec)
    ff = np.maximum(x @ W1 + b1, 0.0) @ W2 + b2
    return ln(ff + x, g2, be2).astype(np.float32)


def _fast_path_ok(i):
    """The Bass program hard-codes all-ones masks, zero biases and identity
    layernorm affines (true for this problem's setup_inputs)."""
    return (np.all(i["mout"] == 1) and np.all(i["min_mask"] == 1)
            and all(np.all(i[k] == 0.0) for k in
                    ("bo_sa", "bo_ca", "b1", "b2", "be1", "bec", "be2"))
            and all(np.all(i[k] == 1.0) for k in ("g1", "gc", "g2")))


_CACHED = {}
LAST_EXEC_NS = None


def kernel(**inputs) -> np.ndarray:
    global LAST_EXEC_NS
    i = {k: np.asarray(v) for k, v in inputs.items()}
    if not _fast_path_ok(i):
        return _np_reference(**i)

    if "nc" not in _CACHED:
        nc_ = _build_program(BPC)
        _split_sync_waits(nc_)
        _CACHED["nc"] = nc_
    nc = _CACHED["nc"]

    f32 = np.float32
    wmap = {
        "wq_sa": i["Wq_sa"], "wk_sa": i["Wk_sa"], "wv_sa": i["Wv_sa"],
        "wo_sa": i["Wo_sa"], "wq_ca": i["Wq_ca"], "wk_ca": i["Wk_ca"],
        "wv_ca": i["Wv_ca"], "wo_ca": i["Wo_ca"],
        "w1": i["W1"], "w2": i["W2"],
    }
    wmap = {k: np.ascontiguousarray(v, dtype=f32) for k, v in wmap.items()}
    x = np.ascontiguousarray(i["x"], dtype=f32)
    enc = np.ascontiguousarray(i["enc_out"], dtype=f32)

    in_maps = []
    for c in range(N_CORES):
        m = dict(wmap)
        m["x"] = x[c * BPC:(c + 1) * BPC]
        m["enc"] = enc[c * BPC:(c + 1) * BPC]
        in_maps.append(m)

    trace = bool(int(os.environ.get("TRN_KERNEL_TRACE", "0")))
    res = bass_utils.run_bass_kernel_spmd(
        nc, in_maps, core_ids=list(range(N_CORES)), trace=trace)
    LAST_EXEC_NS = res.exec_time_ns
    out = np.concatenate([res.results[c]["out"] for c in range(N_CORES)], axis=0)
    return out.astype(i["x"].dtype, copy=False)


# revision 14
# speedup vs baseline: 1.4553x; 1.4553x over previous
"""Trainium2 Bass kernel: transformer decoder block (causal self-attention +
cross-attention + 4x FFN, post-residual layernorms).

Sharding: pure data-parallel over batch. B=64 -> 8 batch elements per core on
8 NeuronCores, no collectives. Each core runs the same Bass program on its
batch shard; weights are replicated.

Layout strategy per core:
  - the residual stream stays fp32 token-major [T_part, C_free] (layernorm /
    residual adds are free-dim reductions there), and is transposed on the PE
    (identity matmul) into fp16 feature-major [C_part, T_free] tiles that feed
    the matmuls. All matmul operands are fp16 (PE runs fp16 at full rate and
    upconverts to fp22 internally; accumulation is fp32 in PSUM), produced by
    converting writes of the PSUM->SBUF eviction ops, so the conversions cost
    nothing extra.
  - attention is computed in S^T layout: S^T[k, q] = K^T.T @ Q^T per head; exp
    runs on the scalar engine straight out of PSUM (max-subtraction is not
    needed: |logits| <= ~6 for layernormed inputs); causal masking is a gpsimd
    affine_select (fill=0 after exp); softmax denominators come from a
    ones-lhsT matmul on the PE that produces 64-row replicated column sums;
    the 1/sum normalization is fused into the O^T PSUM->SBUF eviction.
  - weights are converted to fp16 once at startup and stay SBUF-resident.
"""

import os
import sys
from contextlib import ExitStack

import numpy as np

for _p in ("/opt/trn_rl_repo",):
    if os.path.isdir(_p) and _p not in sys.path:
        sys.path.insert(0, _p)

import concourse.bass as bass
import concourse.tile as tile
from concourse import mybir
from concourse import bass_utils
from concourse.masks import make_identity

B, T, C = 64, 312, 512
NH, HD, FF = 8, 64, 2048
N_CORES = 8
BPC = B // N_CORES
NKC = C // 128          # channel chunks
NFC = FF // 128         # ffn-dim chunks
TT = [(0, 128), (128, 128), (256, T - 256)]   # token tiles (t0, sz)
F32 = mybir.dt.float32
F16 = mybir.dt.float16
AL = mybir.AluOpType
AF = mybir.ActivationFunctionType

_WAIT_CAP = 1


def _split_sync_waits(nc):
    """This walrus build supports only one sync-wait command per instruction.
    Redistribute excess waits onto same-engine nops inserted directly before
    the instruction (waits are pure pre-conditions on monotonic semaphores,
    so hoisting them earlier on the same engine preserves ordering; DMA waits
    execute on the triggering sequencer, so the same argument applies)."""
    cap = _WAIT_CAP
    for bb in nc.main_func.blocks:
        il = bb.instructions
        i = 0
        while i < len(il):
            inst = il[i]
            si = inst.sync_info
            if si is None or not si.on_wait or len(si.on_wait) <= cap:
                i += 1
                continue
            waits = list(si.on_wait)
            extra, keep = waits[:-cap], waits[-cap:]
            inst.sync_info = mybir.SyncInfo(on_wait=keep,
                                            on_update=list(si.on_update or []))
            for j in range(0, len(extra), cap):
                nop = mybir.InstNoOp(name=f"I-waitsplit-{nc.next_id()}",
                                     ins=[], outs=[])
                nop.engine = inst.engine
                nop.sync_info = mybir.SyncInfo(on_wait=extra[j:j + cap],
                                               on_update=[])
                il.insert(i, nop)
                i += 1
            i += 1


def _mm(nc, out, lhsT, rhs, start, stop):
    # skip_group_check: CoreSim's psum group checker mis-tracks partition-
    # sliced accumulation groups (base_partition=64); execution semantics
    # (per-element has_written) are unaffected. Tile still provides ordering.
    nc.tensor.matmul(out, lhsT, rhs, start=start, stop=stop,
                     skip_group_check=True)


def _build_program(bpc):
    """Build the per-core Bass program for `bpc` batch elements."""
    nc = bass.Bass("TRN2", target_bir_lowering=False, debug=False,
                   enable_asserts=False, num_devices=N_CORES)

    xd = nc.dram_tensor("x", [bpc, T, C], F32, kind="ExternalInput").ap()
    ed = nc.dram_tensor("enc", [bpc, T, C], F32, kind="ExternalInput").ap()
    wnames = ["wq_sa", "wk_sa", "wv_sa", "wo_sa",
              "wq_ca", "wk_ca", "wv_ca", "wo_ca"]
    wd = {nm: nc.dram_tensor(nm, [C, C], F32, kind="ExternalInput").ap()
          for nm in wnames}
    w1d = nc.dram_tensor("w1", [C, FF], F32, kind="ExternalInput").ap()
    w2d = nc.dram_tensor("w2", [FF, C], F32, kind="ExternalInput").ap()
    outd = nc.dram_tensor("out", [bpc, T, C], F32, kind="ExternalOutput").ap()

    with tile.TileContext(nc) as tc, ExitStack() as ctx:
        con = ctx.enter_context(tc.tile_pool(name="con", bufs=1))
        stg = ctx.enter_context(tc.tile_pool(name="stg", bufs=2))
        act = ctx.enter_context(tc.tile_pool(name="act", bufs=5))
        actT = ctx.enter_context(tc.tile_pool(name="actT", bufs=4))
        qkp = ctx.enter_context(tc.tile_pool(name="qkp", bufs=2))
        vpp = ctx.enter_context(tc.tile_pool(name="vpp", bufs=2))
        esp = ctx.enter_context(tc.tile_pool(name="esp", bufs=6))
        rbp = ctx.enter_context(tc.tile_pool(name="rbp", bufs=2))
        oTp = ctx.enter_context(tc.tile_pool(name="oTp", bufs=2))
        hTp = ctx.enter_context(tc.tile_pool(name="hTp", bufs=NFC))
        sml = ctx.enter_context(tc.tile_pool(name="sml", bufs=3))
        ps_s = ctx.enter_context(tc.tile_pool(name="ps_s", bufs=2, space="PSUM"))
        ps_o = ctx.enter_context(tc.tile_pool(name="ps_o", bufs=2, space="PSUM"))
        ps_cs = ctx.enter_context(tc.tile_pool(name="ps_cs", bufs=2, space="PSUM"))
        ps_mm = ctx.enter_context(tc.tile_pool(name="ps_mm", bufs=2, space="PSUM"))

        # ---- constants & resident fp16 weights (converted via staging) ----
        ws = {}
        for nm in wnames:
            st = stg.tile([128, NKC, C], F32, name=f"{nm}_st", tag="stg")
            nc.sync.dma_start(out=st, in_=wd[nm].rearrange("(kc p) n -> p kc n", p=128))
            wt = con.tile([128, NKC, C], F16, name=f"{nm}_sb", tag=f"{nm}_sb")
            nc.vector.tensor_copy(wt, st)
            ws[nm] = wt
        w2s = con.tile([128, NFC, C], F16)
        w2r = w2d.rearrange("(fc p) n -> p fc n", p=128)
        for g in range(NFC // NKC):
            st = stg.tile([128, NKC, C], F32, name=f"w2_st{g}", tag="stg")
            nc.sync.dma_start(out=st, in_=w2r[:, g * NKC:(g + 1) * NKC, :])
            nc.vector.tensor_copy(w2s[:, g * NKC:(g + 1) * NKC, :], st)
        w1s = con.tile([128, NKC, FF], F16)
        w1r = w1d.rearrange("(kc p) f -> p kc f", p=128)
        for g in range(NFC // NKC):
            st = stg.tile([128, NKC, C], F32, name=f"w1_st{g}", tag="stg")
            nc.sync.dma_start(out=st, in_=w1r[:, :, g * C:(g + 1) * C])
            nc.vector.tensor_copy(w1s[:, :, g * C:(g + 1) * C], st)
        ident = con.tile([128, 128], F32)
        make_identity(nc, ident)
        ones = con.tile([128, 64], F16)
        nc.vector.memset(ones, 1.0)
        epsT = con.tile([128, 1], F32)
        nc.vector.memset(epsT, 1e-5)

        def transpose_tf(src, nm):
            """fp32 token-major [128, 3, C] -> fp16 feature-major [128, NKC, T]."""
            dst = actT.tile([128, NKC, T], F16, name=nm, tag="actT")
            for cc in range(NKC):
                for it, (t0, sz) in enumerate(TT):
                    tp = ps_s.tile([128, T], F32, name=f"{nm}_tp", tag="s")
                    nc.tensor.transpose(
                        tp[:, :sz], src[:sz, it, cc * 128:(cc + 1) * 128],
                        ident[:sz, :sz])
                    nc.vector.tensor_copy(dst[:, cc, t0:t0 + sz], tp[:, :sz])
            return dst

        def proj_feat(srcT, w, nm):
            """Q^T/K^T-style projection: fp16 [128, NKC, T] = w.T @ srcT."""
            dst = qkp.tile([128, NKC, T], F16, name=nm, tag="qk")
            for mc in range(NKC):
                pp = ps_s.tile([128, T], F32, name=f"{nm}_pp", tag="s")
                for kc in range(NKC):
                    _mm(nc, pp, w[:, kc, mc * 128:(mc + 1) * 128],
                        srcT[:, kc, :], (kc == 0), (kc == NKC - 1))
                nc.vector.tensor_copy(dst[:, mc, :], pp)
            return dst

        def proj_tok(srcT, w, nm):
            """V-style projection, fp16 token-major out [128, 3, C]."""
            dst = vpp.tile([128, 3, C], F16, name=nm, tag="v")
            for it, (t0, sz) in enumerate(TT):
                pp = ps_mm.tile([128, C], F32, name=f"{nm}_pp", tag="mm512")
                for kc in range(NKC):
                    _mm(nc, pp[:sz, :], srcT[:, kc, t0:t0 + sz], w[:, kc, :],
                        (kc == 0), (kc == NKC - 1))
                nc.vector.tensor_copy(dst[:sz, it, :], pp[:sz, :])
            return dst

        def attention(qT, kT, v, causal, nm, filler=None):
            """-> O^T fp16 feature-major [128, NKC, T]: softmax(QK^T/8)V.
            `filler` emits a small chunk of independent PE work after each
            step so the in-order PE stream has something to chew on while
            ACT/gpsimd run the exp/mask chain of this step."""
            oT = oTp.tile([128, NKC, T], F16, name=nm, tag="oT")
            css = rbp.tile([128, NKC, T], F32, name=f"{nm}_css", tag="rb")
            steps = [(h, kt) for h in range(NH) for kt in range(3)]

            def emit_s(h, kt):
                k0, ksz = TT[kt]
                q0 = k0 if causal else 0
                qh = qT[(h % 2) * 64:(h % 2) * 64 + 64, h // 2, :]
                kh = kT[(h % 2) * 64:(h % 2) * 64 + 64, h // 2, :]
                s_ps = ps_s.tile([128, T], F32, name=f"{nm}_s{h}_{kt}", tag="s")
                _mm(nc, s_ps[:ksz, q0:T], kh[:, k0:k0 + ksz], qh[:, q0:T],
                    True, True)
                return s_ps

            s_next = emit_s(*steps[0])
            o_ps = cs_ps = None
            for i, (h, kt) in enumerate(steps):
                pr, half = h // 2, (h % 2) * 64
                osl = slice(half, half + 64)
                k0, ksz = TT[kt]
                q0 = k0 if causal else 0
                s_ps = s_next
                if i + 1 < len(steps):
                    s_next = emit_s(*steps[i + 1])
                es = esp.tile([128, T], F16, name=f"{nm}_es{h}_{kt}", tag="es")
                nc.scalar.activation(es[:ksz, q0:T], s_ps[:ksz, q0:T],
                                     AF.Exp, scale=HD ** -0.5)
                if causal:
                    if k0 > 0:
                        nc.vector.memset(es[:ksz, 0:k0], 0.0)
                    nc.gpsimd.affine_select(
                        out=es[:ksz, k0:T], in_=es[:ksz, k0:T],
                        pattern=[[1, T - k0]], channel_multiplier=-1,
                        base=0, compare_op=AL.is_ge, fill=0.0)
                if kt == 0:
                    o_ps = ps_o.tile([128, T], F32, name=f"{nm}_o{h}", tag="o")
                    cs_ps = ps_cs.tile([128, T], F32, name=f"{nm}_cs{h}", tag="cs")
                _mm(nc, o_ps[osl, :], v[:ksz, kt, h * HD:(h + 1) * HD],
                    es[:ksz, :], (kt == 0), (kt == 2))
                _mm(nc, cs_ps[osl, :], ones[:ksz, :], es[:ksz, :],
                    (kt == 0), (kt == 2))
                if filler is not None:
                    filler()
                if kt == 2:
                    nc.vector.tensor_copy(css[osl, pr, :], cs_ps[osl, :])
                    nc.vector.tensor_copy(oT[osl, pr, :], o_ps[osl, :])
            # one Ln + one Exp(-x) on the scalar engine turn all 8 heads'
            # column sums into reciprocals (2 table loads instead of 8 slow
            # DVE RECIPROCALs), then a single DVE multiply normalizes O^T.
            nc.scalar.activation(css, css, AF.Ln)
            nc.scalar.activation(css, css, AF.Exp, scale=-1.0)
            nc.vector.tensor_tensor(out=oT, in0=oT, in1=css, op=AL.mult)
            return oT

        def out_proj_residual(oT, w, res, nm):
            """fp32 token-major [128, 3, C] = oT.T @ w + res."""
            dst = act.tile([128, 3, C], F32, name=nm, tag="act")
            for it, (t0, sz) in enumerate(TT):
                pp = ps_mm.tile([128, C], F32, name=f"{nm}_pp", tag="mm512")
                for pr in range(NKC):
                    _mm(nc, pp[:sz, :], oT[:, pr, t0:t0 + sz], w[:, pr, :],
                        (pr == 0), (pr == NKC - 1))
                nc.vector.tensor_tensor(out=dst[:sz, it, :], in0=pp[:sz, :],
                                        in1=res[:sz, it, :], op=AL.add)
            return dst

        def layernorm_(r, nm):
            """in-place LN over C on token-major [128, 3, C] (g=1, b=0)."""
            for it, (t0, sz) in enumerate(TT):
                stats = sml.tile([128, 6], F32, name=f"{nm}_st", tag="st")
                nc.vector.bn_stats(out=stats[:sz, :], in_=r[:sz, it, :])
                mv = sml.tile([128, 2], F32, name=f"{nm}_mv", tag="mv")
                nc.vector.bn_aggr(out=mv[:sz, :], in_=stats[:sz, :])
                std = sml.tile([128, 1], F32, name=f"{nm}_sd", tag="sd")
                nc.scalar.activation(std[:sz, :], mv[:sz, 1:2], AF.Sqrt,
                                     bias=epsT[:sz, :])
                rstd = sml.tile([128, 1], F32, name=f"{nm}_rs", tag="rs")
                nc.vector.reciprocal(rstd[:sz, :], std[:sz, :])
                nc.vector.tensor_scalar(
                    out=r[:sz, it, :], in0=r[:sz, it, :],
                    scalar1=mv[:sz, 0:1], scalar2=rstd[:sz, :],
                    op0=AL.subtract, op1=AL.mult)
            return r

        def ffn_thunks(b, x2, x2T):
            """FFN for batch b as a list of small emitters (the cross-batch
            PE gap filler)."""
            st = {"hTs": [], "yp": None}
            th = []

            def mk_h(fc):
                def go():
                    hp = ps_s.tile([128, T], F32, name=f"h{b}_{fc}", tag="s")
                    for kc in range(NKC):
                        _mm(nc, hp, w1s[:, kc, fc * 128:(fc + 1) * 128],
                            x2T[:, kc, :], (kc == 0), (kc == NKC - 1))
                    hT = hTp.tile([128, T], F16, name=f"hT{b}_{fc}", tag="hT")
                    nc.scalar.activation(hT, hp, AF.Relu)
                    st["hTs"].append(hT)
                return go

            for fc in range(NFC):
                th.append(mk_h(fc))
            x3 = act.tile([128, 3, C], F32, name=f"r3_{b}", tag="act")

            def mk_y(it, g):
                def go():
                    t0, sz = TT[it]
                    if g == 0:
                        st["yp"] = ps_mm.tile([128, C], F32,
                                              name=f"y{b}_{it}", tag="mm512")
                    for fc in range(g * 4, g * 4 + 4):
                        _mm(nc, st["yp"][:sz, :], st["hTs"][fc][:, t0:t0 + sz],
                            w2s[:, fc, :], (fc == 0), (fc == NFC - 1))
                return go

            def mk_yev(it):
                def go():
                    t0, sz = TT[it]
                    nc.vector.tensor_tensor(out=x3[:sz, it, :],
                                            in0=st["yp"][:sz, :],
                                            in1=x2[:sz, it, :], op=AL.add)
                return go

            for it in range(3):
                for g in range(NFC // 4):
                    th.append(mk_y(it, g))
                th.append(mk_yev(it))

            def fin():
                layernorm_(x3, f"ln3_{b}")
                for it, (t0, sz) in enumerate(TT):
                    nc.sync.dma_start(out=outd[b, t0:t0 + sz, :],
                                      in_=x3[:sz, it, :])
            th.append(fin)
            return th

        pending = []

        def filler():
            if pending:
                pending.pop(0)()

        for b in range(bpc):
            x_sb = act.tile([128, 3, C], F32, name=f"x{b}", tag="act")
            for it, (t0, sz) in enumerate(TT):
                nc.sync.dma_start(out=x_sb[:sz, it, :], in_=xd[b, t0:t0 + sz, :])
            enc_sb = act.tile([128, 3, C], F32, name=f"e{b}", tag="act")
            for it, (t0, sz) in enumerate(TT):
                nc.sync.dma_start(out=enc_sb[:sz, it, :], in_=ed[b, t0:t0 + sz, :])
            xT = transpose_tf(x_sb, f"xT{b}")
            # ---- self attention ----
            qT = proj_feat(xT, ws["wq_sa"], f"qT{b}")
            kT = proj_feat(xT, ws["wk_sa"], f"kT{b}")
            v = proj_tok(xT, ws["wv_sa"], f"v{b}")
            oT = attention(qT, kT, v, True, f"sa{b}", filler)
            x1 = out_proj_residual(oT, ws["wo_sa"], x_sb, f"r1_{b}")
            # encT is LN1-independent: gives the in-order PE stream ready work
            # while the DVE runs LN1.
            encT = transpose_tf(enc_sb, f"eT{b}")
            filler(); filler()
            layernorm_(x1, f"ln1_{b}")
            # ---- cross attention ----
            x1T = transpose_tf(x1, f"x1T{b}")
            qcT = proj_feat(x1T, ws["wq_ca"], f"qcT{b}")
            kcT = proj_feat(encT, ws["wk_ca"], f"kcT{b}")
            vc = proj_tok(encT, ws["wv_ca"], f"vc{b}")
            oTc = attention(qcT, kcT, vc, False, f"ca{b}", filler)
            x2 = out_proj_residual(oTc, ws["wo_ca"], x1, f"r2_{b}")
            # finish the previous batch's FFN before queueing this one
            while pending:
                pending.pop(0)()
            layernorm_(x2, f"ln2_{b}")
            x2T = transpose_tf(x2, f"x2T{b}")
            pending = ffn_thunks(b, x2, x2T)
        while pending:
            pending.pop(0)()

    return nc


def _np_reference(x, enc_out, min_mask, mout,
                  Wq_sa, Wk_sa, Wv_sa, Wo_sa, bo_sa,
                  Wq_ca, Wk_ca, Wv_ca, Wo_ca, bo_ca,
                  W1, b1, W2, b2, g1, be1, gc, bec, g2, be2):
    """Pure-numpy fallback (exact reference semantics, any inputs)."""
    def ln(x, g, b, eps=1e-5):
        m = x.mean(-1, keepdims=True)
        v = ((x - m) ** 2).mean(-1, keepdims=True)
        return (x - m) / np.sqrt(v + eps) * g + b

    def mha(xq, xkv, Wq, Wk, Wv, Wo, bo, key_mask, causal):
        Bq, Tq, Cc = xq.shape
        Tk = xkv.shape[1]
        q = (xq @ Wq).reshape(Bq, Tq, NH, HD)
        k = (xkv @ Wk).reshape(Bq, Tk, NH, HD)
        vv = (xkv @ Wv).reshape(Bq, Tk, NH, HD)
        wei = np.einsum("bqhd,bkhd->bhqk", q, k) * (HD ** -0.5)
        mask = (key_mask[:, None, None, :] != 0)
        if causal:
            tril = np.tril(np.ones((Tq, Tk), bool))
            mask = mask & tril[None, None]
        wei = np.where(mask, wei, -1e30)
        wei = wei - wei.max(-1, keepdims=True)
        wei = np.exp(wei)
        wei = wei / wei.sum(-1, keepdims=True)
        out = np.einsum("bhqk,bkhd->bqhd", wei, vv).reshape(Bq, Tq, Cc)
        return out @ Wo + bo

    x = x.astype(np.float64)
    att = mha(x, x, Wq_sa, Wk_sa, Wv_sa, Wo_sa, bo_sa, mout, True)
    x = ln(att + x, g1, be1)
    catt = mha(x, enc_out.astype(np.float64), Wq_ca, Wk_ca, Wv_ca, Wo_ca,
               bo_ca, min_mask, False)
    x = ln(catt + x, gc, bec)
    ff = np.maximum(x @ W1 + b1, 0.0) @ W2 + b2
    return ln(ff + x, g2, be2).astype(np.float32)


def _fast_path_ok(i):
    """The Bass program hard-codes all-ones masks, zero biases and identity
    layernorm affines (true for this problem's setup_inputs)."""
    return (np.all(i["mout"] == 1) and np.all(i["min_mask"] == 1)
            and all(np.all(i[k] == 0.0) for k in
                    ("bo_sa", "bo_ca", "b1", "b2", "be1", "bec", "be2"))
            and all(np.all(i[k] == 1.0) for k in ("g1", "gc", "g2")))


_CACHED = {}
LAST_EXEC_NS = None


def kernel(**inputs) -> np.ndarray:
    global LAST_EXEC_NS
    i = {k: np.asarray(v) for k, v in inputs.items()}
    if not _fast_path_ok(i):
        return _np_reference(**i)

    if "nc" not in _CACHED:
        nc_ = _build_program(BPC)
        _split_sync_waits(nc_)
        _CACHED["nc"] = nc_
    nc = _CACHED["nc"]

    f32 = np.float32
    wmap = {
        "wq_sa": i["Wq_sa"], "wk_sa": i["Wk_sa"], "wv_sa": i["Wv_sa"],
        "wo_sa": i["Wo_sa"], "wq_ca": i["Wq_ca"], "wk_ca": i["Wk_ca"],
        "wv_ca": i["Wv_ca"], "wo_ca": i["Wo_ca"],
        "w1": i["W1"], "w2": i["W2"],
    }
    wmap = {k: np.ascontiguousarray(v, dtype=f32) for k, v in wmap.items()}
    x = np.ascontiguousarray(i["x"], dtype=f32)
    enc = np.ascontiguousarray(i["enc_out"], dtype=f32)

    in_maps = []
    for c in range(N_CORES):
        m = dict(wmap)
        m["x"] = x[c * BPC:(c + 1) * BPC]
        m["enc"] = enc[c * BPC:(c + 1) * BPC]
        in_maps.append(m)

    trace = bool(int(os.environ.get("TRN_KERNEL_TRACE", "0")))
    res = bass_utils.run_bass_kernel_spmd(
        nc, in_maps, core_ids=list(range(N_CORES)), trace=trace)
    LAST_EXEC_NS = res.exec_time_ns
    out = np.concatenate([res.results[c]["out"] for c in range(N_CORES)], axis=0)
    return out.astype(i["x"].dtype, copy=False)


# revision 15
# speedup vs baseline: 1.4556x; 1.0002x over previous
"""Trainium2 Bass kernel: transformer decoder block (causal self-attention +
cross-attention + 4x FFN, post-residual layernorms).

Sharding: pure data-parallel over batch. B=64 -> 8 batch elements per core on
8 NeuronCores, no collectives. Each core runs the same Bass program on its
batch shard; weights are replicated.

Layout strategy per core:
  - the residual stream stays fp32 token-major [T_part, C_free] (layernorm /
    residual adds are free-dim reductions there), and is transposed on the PE
    (identity matmul) into fp16 feature-major [C_part, T_free] tiles that feed
    the matmuls. All matmul operands are fp16 (PE runs fp16 at full rate and
    upconverts to fp22 internally; accumulation is fp32 in PSUM), produced by
    converting writes of the PSUM->SBUF eviction ops, so the conversions cost
    nothing extra.
  - attention is computed in S^T layout: S^T[k, q] = K^T.T @ Q^T per head; exp
    runs on the scalar engine straight out of PSUM (max-subtraction is not
    needed: |logits| <= ~6 for layernormed inputs); causal masking is a gpsimd
    affine_select (fill=0 after exp); softmax denominators come from a
    ones-lhsT matmul on the PE that produces 64-row replicated column sums;
    the 1/sum normalization is fused into the O^T PSUM->SBUF eviction.
  - weights are converted to fp16 once at startup and stay SBUF-resident.
"""

import os
import sys
from contextlib import ExitStack

import numpy as np

for _p in ("/opt/trn_rl_repo",):
    if os.path.isdir(_p) and _p not in sys.path:
        sys.path.insert(0, _p)

import concourse.bass as bass
import concourse.tile as tile
from concourse import mybir
from concourse import bass_utils
from concourse.masks import make_identity

B, T, C = 64, 312, 512
NH, HD, FF = 8, 64, 2048
N_CORES = 8
BPC = B // N_CORES
NKC = C // 128          # channel chunks
NFC = FF // 128         # ffn-dim chunks
TT = [(0, 128), (128, 128), (256, T - 256)]   # token tiles (t0, sz)
F32 = mybir.dt.float32
F16 = mybir.dt.float16
AL = mybir.AluOpType
AF = mybir.ActivationFunctionType

_WAIT_CAP = 1


def _split_sync_waits(nc):
    """This walrus build supports only one sync-wait command per instruction.
    Redistribute excess waits onto same-engine nops inserted directly before
    the instruction (waits are pure pre-conditions on monotonic semaphores,
    so hoisting them earlier on the same engine preserves ordering; DMA waits
    execute on the triggering sequencer, so the same argument applies)."""
    cap = _WAIT_CAP
    for bb in nc.main_func.blocks:
        il = bb.instructions
        i = 0
        while i < len(il):
            inst = il[i]
            si = inst.sync_info
            if si is None or not si.on_wait or len(si.on_wait) <= cap:
                i += 1
                continue
            waits = list(si.on_wait)
            extra, keep = waits[:-cap], waits[-cap:]
            inst.sync_info = mybir.SyncInfo(on_wait=keep,
                                            on_update=list(si.on_update or []))
            for j in range(0, len(extra), cap):
                nop = mybir.InstNoOp(name=f"I-waitsplit-{nc.next_id()}",
                                     ins=[], outs=[])
                nop.engine = inst.engine
                nop.sync_info = mybir.SyncInfo(on_wait=extra[j:j + cap],
                                               on_update=[])
                il.insert(i, nop)
                i += 1
            i += 1


def _mm(nc, out, lhsT, rhs, start, stop):
    # skip_group_check: CoreSim's psum group checker mis-tracks partition-
    # sliced accumulation groups (base_partition=64); execution semantics
    # (per-element has_written) are unaffected. Tile still provides ordering.
    nc.tensor.matmul(out, lhsT, rhs, start=start, stop=stop,
                     skip_group_check=True)


def _build_program(bpc):
    """Build the per-core Bass program for `bpc` batch elements."""
    nc = bass.Bass("TRN2", target_bir_lowering=False, debug=False,
                   enable_asserts=False, num_devices=N_CORES)

    xd = nc.dram_tensor("x", [bpc, T, C], F32, kind="ExternalInput").ap()
    ed = nc.dram_tensor("enc", [bpc, T, C], F32, kind="ExternalInput").ap()
    wnames = ["wq_sa", "wk_sa", "wv_sa", "wo_sa",
              "wq_ca", "wk_ca", "wv_ca", "wo_ca"]
    wd = {nm: nc.dram_tensor(nm, [C, C], F32, kind="ExternalInput").ap()
          for nm in wnames}
    w1d = nc.dram_tensor("w1", [C, FF], F32, kind="ExternalInput").ap()
    w2d = nc.dram_tensor("w2", [FF, C], F32, kind="ExternalInput").ap()
    outd = nc.dram_tensor("out", [bpc, T, C], F32, kind="ExternalOutput").ap()

    with tile.TileContext(nc) as tc, ExitStack() as ctx:
        con = ctx.enter_context(tc.tile_pool(name="con", bufs=1))
        stg = ctx.enter_context(tc.tile_pool(name="stg", bufs=2))
        act = ctx.enter_context(tc.tile_pool(name="act", bufs=5))
        actT = ctx.enter_context(tc.tile_pool(name="actT", bufs=4))
        qkp = ctx.enter_context(tc.tile_pool(name="qkp", bufs=2))
        vpp = ctx.enter_context(tc.tile_pool(name="vpp", bufs=2))
        esp = ctx.enter_context(tc.tile_pool(name="esp", bufs=6))
        rbp = ctx.enter_context(tc.tile_pool(name="rbp", bufs=2))
        oTp = ctx.enter_context(tc.tile_pool(name="oTp", bufs=2))
        hTp = ctx.enter_context(tc.tile_pool(name="hTp", bufs=NFC))
        sml = ctx.enter_context(tc.tile_pool(name="sml", bufs=3))
        ps_s = ctx.enter_context(tc.tile_pool(name="ps_s", bufs=2, space="PSUM"))
        ps_o = ctx.enter_context(tc.tile_pool(name="ps_o", bufs=2, space="PSUM"))
        ps_cs = ctx.enter_context(tc.tile_pool(name="ps_cs", bufs=1, space="PSUM"))
        ps_h = ctx.enter_context(tc.tile_pool(name="ps_h", bufs=1, space="PSUM"))
        ps_mm = ctx.enter_context(tc.tile_pool(name="ps_mm", bufs=2, space="PSUM"))

        # ---- constants & resident fp16 weights (converted via staging) ----
        ws = {}
        for nm in wnames:
            st = stg.tile([128, NKC, C], F32, name=f"{nm}_st", tag="stg")
            nc.sync.dma_start(out=st, in_=wd[nm].rearrange("(kc p) n -> p kc n", p=128))
            wt = con.tile([128, NKC, C], F16, name=f"{nm}_sb", tag=f"{nm}_sb")
            nc.vector.tensor_copy(wt, st)
            ws[nm] = wt
        w2s = con.tile([128, NFC, C], F16)
        w2r = w2d.rearrange("(fc p) n -> p fc n", p=128)
        for g in range(NFC // NKC):
            st = stg.tile([128, NKC, C], F32, name=f"w2_st{g}", tag="stg")
            nc.sync.dma_start(out=st, in_=w2r[:, g * NKC:(g + 1) * NKC, :])
            nc.vector.tensor_copy(w2s[:, g * NKC:(g + 1) * NKC, :], st)
        w1s = con.tile([128, NKC, FF], F16)
        w1r = w1d.rearrange("(kc p) f -> p kc f", p=128)
        for g in range(NFC // NKC):
            st = stg.tile([128, NKC, C], F32, name=f"w1_st{g}", tag="stg")
            nc.sync.dma_start(out=st, in_=w1r[:, :, g * C:(g + 1) * C])
            nc.vector.tensor_copy(w1s[:, :, g * C:(g + 1) * C], st)
        ident = con.tile([128, 128], F32)
        make_identity(nc, ident)
        ones = con.tile([128, 64], F16)
        nc.vector.memset(ones, 1.0)
        epsT = con.tile([128, 1], F32)
        nc.vector.memset(epsT, 1e-5)

        def transpose_tf(src, nm):
            """fp32 token-major [128, 3, C] -> fp16 feature-major [128, NKC, T]."""
            dst = actT.tile([128, NKC, T], F16, name=nm, tag="actT")
            for cc in range(NKC):
                for it, (t0, sz) in enumerate(TT):
                    tp = ps_s.tile([128, T], F32, name=f"{nm}_tp", tag="s")
                    nc.tensor.transpose(
                        tp[:, :sz], src[:sz, it, cc * 128:(cc + 1) * 128],
                        ident[:sz, :sz])
                    nc.vector.tensor_copy(dst[:, cc, t0:t0 + sz], tp[:, :sz])
            return dst

        def proj_feat(srcT, w, nm):
            """Q^T/K^T-style projection: fp16 [128, NKC, T] = w.T @ srcT."""
            dst = qkp.tile([128, NKC, T], F16, name=nm, tag="qk")
            for mc in range(NKC):
                pp = ps_s.tile([128, T], F32, name=f"{nm}_pp", tag="s")
                for kc in range(NKC):
                    _mm(nc, pp, w[:, kc, mc * 128:(mc + 1) * 128],
                        srcT[:, kc, :], (kc == 0), (kc == NKC - 1))
                nc.vector.tensor_copy(dst[:, mc, :], pp)
            return dst

        def proj_tok(srcT, w, nm):
            """V-style projection, fp16 token-major out [128, 3, C]."""
            dst = vpp.tile([128, 3, C], F16, name=nm, tag="v")
            for it, (t0, sz) in enumerate(TT):
                pp = ps_mm.tile([128, C], F32, name=f"{nm}_pp", tag="mm512")
                for kc in range(NKC):
                    _mm(nc, pp[:sz, :], srcT[:, kc, t0:t0 + sz], w[:, kc, :],
                        (kc == 0), (kc == NKC - 1))
                nc.vector.tensor_copy(dst[:sz, it, :], pp[:sz, :])
            return dst

        def attention(qT, kT, v, causal, nm, filler=None):
            """-> O^T fp16 feature-major [128, NKC, T]: softmax(QK^T/8)V.
            `filler` emits a small chunk of independent PE work after each
            step so the in-order PE stream has something to chew on while
            ACT/gpsimd run the exp/mask chain of this step."""
            oT = oTp.tile([128, NKC, T], F16, name=nm, tag="oT")
            css = rbp.tile([128, NKC, T], F32, name=f"{nm}_css", tag="rb")
            steps = [(h, kt) for h in range(NH) for kt in range(3)]

            def emit_s(h, kt):
                k0, ksz = TT[kt]
                q0 = k0 if causal else 0
                qh = qT[(h % 2) * 64:(h % 2) * 64 + 64, h // 2, :]
                kh = kT[(h % 2) * 64:(h % 2) * 64 + 64, h // 2, :]
                s_ps = ps_s.tile([128, T], F32, name=f"{nm}_s{h}_{kt}", tag="s")
                _mm(nc, s_ps[:ksz, q0:T], kh[:, k0:k0 + ksz], qh[:, q0:T],
                    True, True)
                return s_ps

            s_next = emit_s(*steps[0])
            o_ps = cs_ps = None
            for i, (h, kt) in enumerate(steps):
                pr, half = h // 2, (h % 2) * 64
                osl = slice(half, half + 64)
                k0, ksz = TT[kt]
                q0 = k0 if causal else 0
                s_ps = s_next
                if i + 1 < len(steps):
                    s_next = emit_s(*steps[i + 1])
                es = esp.tile([128, T], F16, name=f"{nm}_es{h}_{kt}", tag="es")
                nc.scalar.activation(es[:ksz, q0:T], s_ps[:ksz, q0:T],
                                     AF.Exp, scale=HD ** -0.5)
                if causal:
                    if k0 > 0:
                        nc.vector.memset(es[:ksz, 0:k0], 0.0)
                    nc.gpsimd.affine_select(
                        out=es[:ksz, k0:T], in_=es[:ksz, k0:T],
                        pattern=[[1, T - k0]], channel_multiplier=-1,
                        base=0, compare_op=AL.is_ge, fill=0.0)
                if kt == 0:
                    o_ps = ps_o.tile([128, T], F32, name=f"{nm}_o{h}", tag="o")
                    cs_ps = ps_cs.tile([128, T], F32, name=f"{nm}_cs{h}", tag="cs")
                _mm(nc, o_ps[osl, :], v[:ksz, kt, h * HD:(h + 1) * HD],
                    es[:ksz, :], (kt == 0), (kt == 2))
                _mm(nc, cs_ps[osl, :], ones[:ksz, :], es[:ksz, :],
                    (kt == 0), (kt == 2))
                if filler is not None:
                    filler()
                if kt == 2:
                    nc.vector.tensor_copy(css[osl, pr, :], cs_ps[osl, :])
                    nc.vector.tensor_copy(oT[osl, pr, :], o_ps[osl, :])
            # one Ln + one Exp(-x) on the scalar engine turn all 8 heads'
            # column sums into reciprocals (2 table loads instead of 8 slow
            # DVE RECIPROCALs), then a single DVE multiply normalizes O^T.
            nc.scalar.activation(css, css, AF.Ln)
            nc.scalar.activation(css, css, AF.Exp, scale=-1.0)
            nc.vector.tensor_tensor(out=oT, in0=oT, in1=css, op=AL.mult)
            return oT

        def out_proj_residual(oT, w, res, nm):
            """fp32 token-major [128, 3, C] = oT.T @ w + res."""
            dst = act.tile([128, 3, C], F32, name=nm, tag="act")
            for it, (t0, sz) in enumerate(TT):
                pp = ps_mm.tile([128, C], F32, name=f"{nm}_pp", tag="mm512")
                for pr in range(NKC):
                    _mm(nc, pp[:sz, :], oT[:, pr, t0:t0 + sz], w[:, pr, :],
                        (pr == 0), (pr == NKC - 1))
                nc.vector.tensor_tensor(out=dst[:sz, it, :], in0=pp[:sz, :],
                                        in1=res[:sz, it, :], op=AL.add)
            return dst

        def layernorm_(r, nm):
            """in-place LN over C on token-major [128, 3, C] (g=1, b=0)."""
            for it, (t0, sz) in enumerate(TT):
                stats = sml.tile([128, 6], F32, name=f"{nm}_st", tag="st")
                nc.vector.bn_stats(out=stats[:sz, :], in_=r[:sz, it, :])
                mv = sml.tile([128, 2], F32, name=f"{nm}_mv", tag="mv")
                nc.vector.bn_aggr(out=mv[:sz, :], in_=stats[:sz, :])
                std = sml.tile([128, 1], F32, name=f"{nm}_sd", tag="sd")
                nc.scalar.activation(std[:sz, :], mv[:sz, 1:2], AF.Sqrt,
                                     bias=epsT[:sz, :])
                rstd = sml.tile([128, 1], F32, name=f"{nm}_rs", tag="rs")
                nc.vector.reciprocal(rstd[:sz, :], std[:sz, :])
                nc.vector.tensor_scalar(
                    out=r[:sz, it, :], in0=r[:sz, it, :],
                    scalar1=mv[:sz, 0:1], scalar2=rstd[:sz, :],
                    op0=AL.subtract, op1=AL.mult)
            return r

        def ffn_thunks(b, x2, x2T):
            """FFN for batch b as a list of small emitters (the cross-batch
            PE gap filler)."""
            st = {"hTs": [], "yp": None}
            th = []

            def mk_h(fc):
                def go():
                    hp = ps_h.tile([128, T], F32, name=f"h{b}_{fc}", tag="h")
                    for kc in range(NKC):
                        _mm(nc, hp, w1s[:, kc, fc * 128:(fc + 1) * 128],
                            x2T[:, kc, :], (kc == 0), (kc == NKC - 1))
                    hT = hTp.tile([128, T], F16, name=f"hT{b}_{fc}", tag="hT")
                    nc.scalar.activation(hT, hp, AF.Relu)
                    st["hTs"].append(hT)
                return go

            for fc in range(NFC):
                th.append(mk_h(fc))
            x3 = act.tile([128, 3, C], F32, name=f"r3_{b}", tag="act")

            def mk_y(it, g):
                def go():
                    t0, sz = TT[it]
                    if g == 0:
                        st["yp"] = ps_mm.tile([128, C], F32,
                                              name=f"y{b}_{it}", tag="mm512")
                    for fc in range(g * 4, g * 4 + 4):
                        _mm(nc, st["yp"][:sz, :], st["hTs"][fc][:, t0:t0 + sz],
                            w2s[:, fc, :], (fc == 0), (fc == NFC - 1))
                return go

            def mk_yev(it):
                def go():
                    t0, sz = TT[it]
                    nc.vector.tensor_tensor(out=x3[:sz, it, :],
                                            in0=st["yp"][:sz, :],
                                            in1=x2[:sz, it, :], op=AL.add)
                return go

            for it in range(3):
                for g in range(NFC // 4):
                    th.append(mk_y(it, g))
                th.append(mk_yev(it))

            def fin():
                layernorm_(x3, f"ln3_{b}")
                for it, (t0, sz) in enumerate(TT):
                    nc.sync.dma_start(out=outd[b, t0:t0 + sz, :],
                                      in_=x3[:sz, it, :])
            th.append(fin)
            return th

        pending = []

        def filler():
            if pending:
                pending.pop(0)()

        for b in range(bpc):
            x_sb = act.tile([128, 3, C], F32, name=f"x{b}", tag="act")
            for it, (t0, sz) in enumerate(TT):
                nc.sync.dma_start(out=x_sb[:sz, it, :], in_=xd[b, t0:t0 + sz, :])
            enc_sb = act.tile([128, 3, C], F32, name=f"e{b}", tag="act")
            for it, (t0, sz) in enumerate(TT):
                nc.sync.dma_start(out=enc_sb[:sz, it, :], in_=ed[b, t0:t0 + sz, :])
            xT = transpose_tf(x_sb, f"xT{b}")
            # ---- self attention ----
            qT = proj_feat(xT, ws["wq_sa"], f"qT{b}")
            kT = proj_feat(xT, ws["wk_sa"], f"kT{b}")
            v = proj_tok(xT, ws["wv_sa"], f"v{b}")
            oT = attention(qT, kT, v, True, f"sa{b}", filler)
            x1 = out_proj_residual(oT, ws["wo_sa"], x_sb, f"r1_{b}")
            # encT is LN1-independent: gives the in-order PE stream ready work
            # while the DVE runs LN1.
            encT = transpose_tf(enc_sb, f"eT{b}")
            filler(); filler()
            layernorm_(x1, f"ln1_{b}")
            # ---- cross attention ----
            x1T = transpose_tf(x1, f"x1T{b}")
            qcT = proj_feat(x1T, ws["wq_ca"], f"qcT{b}")
            kcT = proj_feat(encT, ws["wk_ca"], f"kcT{b}")
            vc = proj_tok(encT, ws["wv_ca"], f"vc{b}")
            oTc = attention(qcT, kcT, vc, False, f"ca{b}", filler)
            x2 = out_proj_residual(oTc, ws["wo_ca"], x1, f"r2_{b}")
            # finish the previous batch's FFN before queueing this one
            while pending:
                pending.pop(0)()
            layernorm_(x2, f"ln2_{b}")
            x2T = transpose_tf(x2, f"x2T{b}")
            pending = ffn_thunks(b, x2, x2T)
        while pending:
            pending.pop(0)()

    return nc


def _np_reference(x, enc_out, min_mask, mout,
                  Wq_sa, Wk_sa, Wv_sa, Wo_sa, bo_sa,
                  Wq_ca, Wk_ca, Wv_ca, Wo_ca, bo_ca,
                  W1, b1, W2, b2, g1, be1, gc, bec, g2, be2):
    """Pure-numpy fallback (exact reference semantics, any inputs)."""
    def ln(x, g, b, eps=1e-5):
        m = x.mean(-1, keepdims=True)
        v = ((x - m) ** 2).mean(-1, keepdims=True)
        return (x - m) / np.sqrt(v + eps) * g + b

    def mha(xq, xkv, Wq, Wk, Wv, Wo, bo, key_mask, causal):
        Bq, Tq, Cc = xq.shape
        Tk = xkv.shape[1]
        q = (xq @ Wq).reshape(Bq, Tq, NH, HD)
        k = (xkv @ Wk).reshape(Bq, Tk, NH, HD)
        vv = (xkv @ Wv).reshape(Bq, Tk, NH, HD)
        wei = np.einsum("bqhd,bkhd->bhqk", q, k) * (HD ** -0.5)
        mask = (key_mask[:, None, None, :] != 0)
        if causal:
            tril = np.tril(np.ones((Tq, Tk), bool))
            mask = mask & tril[None, None]
        wei = np.where(mask, wei, -1e30)
        wei = wei - wei.max(-1, keepdims=True)
        wei = np.exp(wei)
        wei = wei / wei.sum(-1, keepdims=True)
        out = np.einsum("bhqk,bkhd->bqhd", wei, vv).reshape(Bq, Tq, Cc)
        return out @ Wo + bo

    x = x.astype(np.float64)
    att = mha(x, x, Wq_sa, Wk_sa, Wv_sa, Wo_sa, bo_sa, mout, True)
    x = ln(att + x, g1, be1)
    catt = mha(x, enc_out.astype(np.float64), Wq_ca, Wk_ca, Wv_ca, Wo_ca,
               bo_ca, min_mask, False)
    x = ln(catt + x, gc, bec)
    ff = np.maximum(x @ W1 + b1, 0.0) @ W2 + b2
    return ln(ff + x, g2, be2).astype(np.float32)


def _fast_path_ok(i):
    """The Bass program hard-codes all-ones masks, zero biases and identity
    layernorm affines (true for this problem's setup_inputs)."""
    return (np.all(i["mout"] == 1) and np.all(i["min_mask"] == 1)
            and all(np.all(i[k] == 0.0) for k in
                    ("bo_sa", "bo_ca", "b1", "b2", "be1", "bec", "be2"))
            and all(np.all(i[k] == 1.0) for k in ("g1", "gc", "g2")))


_CACHED = {}
LAST_EXEC_NS = None


def kernel(**inputs) -> np.ndarray:
    global LAST_EXEC_NS
    i = {k: np.asarray(v) for k, v in inputs.items()}
    if not _fast_path_ok(i):
        return _np_reference(**i)

    if "nc" not in _CACHED:
        nc_ = _build_program(BPC)
        _split_sync_waits(nc_)
        _CACHED["nc"] = nc_
    nc = _CACHED["nc"]

    f32 = np.float32
    wmap = {
        "wq_sa": i["Wq_sa"], "wk_sa": i["Wk_sa"], "wv_sa": i["Wv_sa"],
        "wo_sa": i["Wo_sa"], "wq_ca": i["Wq_ca"], "wk_ca": i["Wk_ca"],
        "wv_ca": i["Wv_ca"], "wo_ca": i["Wo_ca"],
        "w1": i["W1"], "w2": i["W2"],
    }
    wmap = {k: np.ascontiguousarray(v, dtype=f32) for k, v in wmap.items()}
    x = np.ascontiguousarray(i["x"], dtype=f32)
    enc = np.ascontiguousarray(i["enc_out"], dtype=f32)

    in_maps = []
    for c in range(N_CORES):
        m = dict(wmap)
        m["x"] = x[c * BPC:(c + 1) * BPC]
        m["enc"] = enc[c * BPC:(c + 1) * BPC]
        in_maps.append(m)

    trace = bool(int(os.environ.get("TRN_KERNEL_TRACE", "0")))
    res = bass_utils.run_bass_kernel_spmd(
        nc, in_maps, core_ids=list(range(N_CORES)), trace=trace)
    LAST_EXEC_NS = res.exec_time_ns
    out = np.concatenate([res.results[c]["out"] for c in range(N_CORES)], axis=0)
    return out.astype(i["x"].dtype, copy=False)
